# revision 2
# baseline (speedup 1.0000x reference)
"""Trainium2 Bass kernel for nn_LyotFilter: out = x @ w_norm(weight_).

Strategy (data-parallel over 8 NeuronCores):
  - Host: compute the tiny [200, 64] normalized filter matrix, cast x and the
    filter to fp16, and reshape each core's row-shard of x into a transposed,
    contiguous [200, 62500] layout so the contraction dim (200 features) lands
    on SBUF partitions with fully contiguous per-partition DMA.
  - Device (per core): process two pixel half-ranges (columns [0,31250) and
    [31250,62500)) in lockstep.  TensorE computes each half's [64, n] output
    with the PE column-tiling feature (tile_position col 0 / col 64) so both
    halves accumulate into one [128, n] PSUM tile; DVE downcasts PSUM fp32 ->
    SBUF fp16; the output DMA then moves full 128-partition tiles, which
    spreads bytes evenly over all 16 SDMA engines (a 64-partition store would
    load only the even engines at half aggregate bandwidth).
  - HBM traffic/core: 25 MB in + 8 MB out (fp16) ~= 33 MB against a ~358 GB/s
    per-core HBM limit -> ~92 us floor.
  - Host: unstack the two halves of each core's [128, 31250] result,
    concatenate the 8 cores and transpose to [500000, 64] fp32.
"""

import functools

import numpy as np

N_CORES = 8
ROWS = 500000
RPC = ROWS // N_CORES  # 62500 rows per core
IN_DIM = 200
OUT_DIM = 64
K1 = 128               # first contraction chunk (partition limit)
K2 = IN_DIM - K1       # 72
HALF = RPC // 2        # 31250 columns per pixel half-range
F_CHUNK = 6250         # columns of xT per DMA chunk (5 chunks/half)
INNER = 500            # matmul moving free dim (<=512 for fp32 PSUM bank)


def _w_norm(weight_: np.ndarray) -> np.ndarray:
    """[200, 64] filter matrix, float32 arithmetic mimicking the reference."""
    n = np.arange(220)
    skip = ((n >= 103) & (n <= 107)) | ((n >= 149) & (n <= 162)) | (n == 219)
    kept = n[~skip]
    bands = (400.0 + (2500.0 - 400.0) * kept / 220.0).astype(np.float32)
    num = np.float32(2.0 * np.pi * (-0.01))
    denom = weight_.astype(np.float32)[:, None] * (bands[None, :] * np.float32(1e-6))
    phase = (num / denom).astype(np.float32)
    w = np.float32(0.5) - np.float32(0.5) * np.cos(phase)
    wt = w.T  # [200, 64]
    wn = np.maximum(wt, np.float32(0.0)) / wt.sum(axis=0, dtype=np.float32)
    return np.ascontiguousarray(wn.astype(np.float32))


@functools.cache
def _build():
    import concourse.bass as bass
    import concourse.tile as tile
    from concourse import bacc, mybir

    f32 = mybir.dt.float32
    f16 = mybir.dt.float16
    nc = bacc.Bacc(
        "TRN2", target_bir_lowering=False, debug=False, num_devices=N_CORES
    )
    xt = nc.declare_dram_parameter("xt", [IN_DIM, RPC], f16, isOutput=False)
    wn = nc.declare_dram_parameter("wn", [IN_DIM, OUT_DIM], f16, isOutput=False)
    out = nc.declare_dram_parameter("out2", [2 * OUT_DIM, HALF], f16, isOutput=True)

    with tile.TileContext(nc) as tc:
        with (
            tc.tile_pool(name="w", bufs=1) as wp,
            tc.tile_pool(name="x1a", bufs=3) as p1a,
            tc.tile_pool(name="x2a", bufs=2) as p2a,
            tc.tile_pool(name="x1b", bufs=3) as p1b,
            tc.tile_pool(name="x2b", bufs=2) as p2b,
            tc.tile_pool(name="outp", bufs=3) as po,
            tc.tile_pool(name="ps", bufs=6, space=bass.MemorySpace.PSUM) as pp,
        ):
            w1 = wp.tile([K1, OUT_DIM], f16, tag="w1")
            w2 = wp.tile([K2, OUT_DIM], f16, tag="w2")
            nc.sync.dma_start(w1[:], wn[0:K1, :])
            nc.scalar.dma_start(w2[:], wn[K1:IN_DIM, :])

            for f0 in range(0, HALF, F_CHUNK):
                fs = min(F_CHUNK, HALF - f0)
                t1a = p1a.tile([K1, F_CHUNK], f16, tag="x1a")
                t2a = p2a.tile([K2, F_CHUNK], f16, tag="x2a")
                t1b = p1b.tile([K1, F_CHUNK], f16, tag="x1b")
                t2b = p2b.tile([K2, F_CHUNK], f16, tag="x2b")
                # halves ride separate HWDGE rings: bytes balance exactly
                nc.sync.dma_start(t1a[:, :fs], xt[0:K1, f0 : f0 + fs])
                nc.sync.dma_start(t2a[:, :fs], xt[K1:IN_DIM, f0 : f0 + fs])
                g0 = HALF + f0
                nc.scalar.dma_start(t1b[:, :fs], xt[0:K1, g0 : g0 + fs])
                nc.scalar.dma_start(t2b[:, :fs], xt[K1:IN_DIM, g0 : g0 + fs])

                ot = po.tile([2 * OUT_DIM, F_CHUNK], f16, tag="out")
                j = 0
                while j < fs:
                    nn = min(INNER, fs - j)
                    ps = pp.tile([2 * OUT_DIM, INNER], f32, tag="ps")
                    nc.tensor.matmul(
                        ps[0:OUT_DIM, :nn], w1[:], t1a[:, j : j + nn],
                        start=True, stop=False,
                    )
                    nc.tensor.matmul(
                        ps[0:OUT_DIM, :nn], w2[:], t2a[:, j : j + nn],
                        start=False, stop=True,
                    )
                    nc.tensor.matmul(
                        ps[OUT_DIM:, :nn], w1[:], t1b[:, j : j + nn],
                        start=True, stop=False,
                    )
                    nc.tensor.matmul(
                        ps[OUT_DIM:, :nn], w2[:], t2b[:, j : j + nn],
                        start=False, stop=True,
                    )
                    nc.vector.tensor_copy(ot[:, j : j + nn], ps[:, :nn])
                    j += nn
                # outputs ride the SWDGE ring so they don't head-of-line
                # block the next chunk's input loads
                nc.gpsimd.dma_start(out[:, f0 : f0 + fs], ot[:, :fs])
    nc.compile()
    return nc


def _run(in_maps, trace=False, **kw):
    from concourse.bass_utils import run_bass_kernel_spmd

    nc = _build()
    return run_bass_kernel_spmd(nc, in_maps, list(range(N_CORES)), trace=trace, **kw)


def _make_in_maps(x: np.ndarray, weight_: np.ndarray):
    wn = _w_norm(weight_).astype(np.float16)
    in_maps = []
    for i in range(N_CORES):
        xti = np.ascontiguousarray(x[i * RPC : (i + 1) * RPC, :].T.astype(np.float16))
        in_maps.append({"xt": xti, "wn": wn})
    return in_maps


def _assemble(results) -> np.ndarray:
    """[128, 31250] per-core results -> [500000, 64] fp32."""
    parts = []
    for i in range(N_CORES):
        o2 = results[i]["out2"]
        parts.append(o2[0:OUT_DIM, :])
        parts.append(o2[OUT_DIM:, :])
    out_t = np.concatenate(parts, axis=1)  # [64, 500000]
    return np.ascontiguousarray(out_t.T).astype(np.float32)


def kernel(x: np.ndarray, weight_: np.ndarray) -> np.ndarray:
    x = np.asarray(x)
    weight_ = np.asarray(weight_)
    res = _run(_make_in_maps(x, weight_)).results
    return _assemble(res)


# revision 5
# speedup vs baseline: 1.0915x; 1.0915x over previous
"""Trainium2 Bass kernel for nn_LyotFilter: out = x @ w_norm(weight_).

Strategy (data-parallel over 8 NeuronCores):
  - Host: compute the tiny [200, 64] normalized filter matrix, cast x and the
    filter to fp16, and reshape each core's row-shard of x into a transposed,
    contiguous [200, 62500] layout so the contraction dim (200 features) lands
    on SBUF partitions with fully contiguous per-partition DMA.
  - Device (per core): process two pixel half-ranges (columns [0,31250) and
    [31250,62500)) in lockstep.  TensorE computes each half's [64, n] output
    with the PE column-tiling feature (tile_position col 0 / col 64) so both
    halves accumulate into one [128, n] PSUM tile; DVE downcasts PSUM fp32 ->
    SBUF fp16; the output DMA then moves full 128-partition tiles, which
    spreads bytes evenly over all 16 SDMA engines (a 64-partition store would
    load only the even engines at half aggregate bandwidth).
  - HBM traffic/core: 25 MB in + 8 MB out (fp16) ~= 33 MB against a ~358 GB/s
    per-core HBM limit -> ~92 us floor.
  - Host: unstack the two halves of each core's [128, 31250] result,
    concatenate the 8 cores and transpose to [500000, 64] fp32.
"""

import functools

import numpy as np

N_CORES = 8
ROWS = 500000
RPC = ROWS // N_CORES  # 62500 rows per core
IN_DIM = 200
OUT_DIM = 64
K1 = 128               # first contraction chunk (partition limit)
K2 = IN_DIM - K1       # 72
HALF = RPC // 2        # 31250 columns per pixel half-range
# tapered chunk plan: small first chunk gets output traffic flowing early,
# small last chunk shrinks the post-input drain tail
CHUNKS = [3125, 6250, 6250, 6250, 6250, 3125]
INNER = 500            # matmul moving free dim (<=512 for fp32 PSUM bank)


def _w_norm(weight_: np.ndarray) -> np.ndarray:
    """[200, 64] filter matrix, float32 arithmetic mimicking the reference."""
    n = np.arange(220)
    skip = ((n >= 103) & (n <= 107)) | ((n >= 149) & (n <= 162)) | (n == 219)
    kept = n[~skip]
    bands = (400.0 + (2500.0 - 400.0) * kept / 220.0).astype(np.float32)
    num = np.float32(2.0 * np.pi * (-0.01))
    denom = weight_.astype(np.float32)[:, None] * (bands[None, :] * np.float32(1e-6))
    phase = (num / denom).astype(np.float32)
    w = np.float32(0.5) - np.float32(0.5) * np.cos(phase)
    wt = w.T  # [200, 64]
    wn = np.maximum(wt, np.float32(0.0)) / wt.sum(axis=0, dtype=np.float32)
    return np.ascontiguousarray(wn.astype(np.float32))


@functools.cache
def _build():
    import concourse.bass as bass
    import concourse.tile as tile
    from concourse import bacc, mybir

    f32 = mybir.dt.float32
    f16 = mybir.dt.float16
    nc = bacc.Bacc(
        "TRN2", target_bir_lowering=False, debug=False, num_devices=N_CORES
    )
    xt = nc.declare_dram_parameter("xt", [IN_DIM, RPC], f16, isOutput=False)
    wn = nc.declare_dram_parameter("wn", [IN_DIM, OUT_DIM], f16, isOutput=False)
    out = nc.declare_dram_parameter("out2", [2 * OUT_DIM, HALF], f16, isOutput=True)

    with tile.TileContext(nc) as tc:
        with (
            tc.tile_pool(name="w", bufs=1) as wp,
            tc.tile_pool(name="x1a", bufs=3) as p1a,
            tc.tile_pool(name="x2a", bufs=2) as p2a,
            tc.tile_pool(name="x1b", bufs=3) as p1b,
            tc.tile_pool(name="x2b", bufs=2) as p2b,
            tc.tile_pool(name="outp", bufs=3) as po,
            tc.tile_pool(name="ps", bufs=8, space=bass.MemorySpace.PSUM) as pp,
        ):
            w1 = wp.tile([K1, OUT_DIM], f16, tag="w1")
            w2 = wp.tile([K2, OUT_DIM], f16, tag="w2")
            # weights ride SWDGE so the HWDGE rings start on x immediately
            nc.gpsimd.dma_start(w1[:], wn[0:K1, :])
            nc.gpsimd.dma_start(w2[:], wn[K1:IN_DIM, :])

            F_MAX = max(CHUNKS)
            f0 = 0
            for fs in CHUNKS:
                t1a = p1a.tile([K1, F_MAX], f16, tag="x1a")
                t2a = p2a.tile([K2, F_MAX], f16, tag="x2a")
                t1b = p1b.tile([K1, F_MAX], f16, tag="x1b")
                t2b = p2b.tile([K2, F_MAX], f16, tag="x2b")
                # halves ride separate HWDGE rings: bytes balance exactly
                nc.sync.dma_start(t1a[:, :fs], xt[0:K1, f0 : f0 + fs])
                nc.sync.dma_start(t2a[:, :fs], xt[K1:IN_DIM, f0 : f0 + fs])
                g0 = HALF + f0
                nc.scalar.dma_start(t1b[:, :fs], xt[0:K1, g0 : g0 + fs])
                nc.scalar.dma_start(t2b[:, :fs], xt[K1:IN_DIM, g0 : g0 + fs])

                ot = po.tile([2 * OUT_DIM, F_MAX], f16, tag="out")
                # split the store at a subtile boundary near the middle so
                # output bytes start draining mid-chunk instead of all at once
                jmid = (fs // 2 + INNER - 1) // INNER * INNER
                j = 0
                while j < fs:
                    nn = min(INNER, fs - j)
                    ps = pp.tile([2 * OUT_DIM, INNER], f32, tag="ps")
                    nc.tensor.matmul(
                        ps[0:OUT_DIM, :nn], w1[:], t1a[:, j : j + nn],
                        start=True, stop=False,
                    )
                    nc.tensor.matmul(
                        ps[0:OUT_DIM, :nn], w2[:], t2a[:, j : j + nn],
                        start=False, stop=True,
                    )
                    nc.tensor.matmul(
                        ps[OUT_DIM:, :nn], w1[:], t1b[:, j : j + nn],
                        start=True, stop=False,
                    )
                    nc.tensor.matmul(
                        ps[OUT_DIM:, :nn], w2[:], t2b[:, j : j + nn],
                        start=False, stop=True,
                    )
                    nc.vector.tensor_copy(ot[:, j : j + nn], ps[:, :nn])
                    j += nn
                    if j == jmid:
                        # outputs ride the SWDGE ring so they don't
                        # head-of-line block the next chunk's input loads
                        nc.gpsimd.dma_start(out[:, f0 : f0 + jmid], ot[:, :jmid])
                nc.gpsimd.dma_start(
                    out[:, f0 + jmid : f0 + fs], ot[:, jmid:fs]
                )
                f0 += fs
    nc.compile()
    return nc


def _run(in_maps, trace=False, **kw):
    from concourse.bass_utils import run_bass_kernel_spmd

    nc = _build()
    return run_bass_kernel_spmd(nc, in_maps, list(range(N_CORES)), trace=trace, **kw)


def _make_in_maps(x: np.ndarray, weight_: np.ndarray):
    wn = _w_norm(weight_).astype(np.float16)
    in_maps = []
    for i in range(N_CORES):
        xti = np.ascontiguousarray(x[i * RPC : (i + 1) * RPC, :].T.astype(np.float16))
        in_maps.append({"xt": xti, "wn": wn})
    return in_maps


def _assemble(results) -> np.ndarray:
    """[128, 31250] per-core results -> [500000, 64] fp32."""
    parts = []
    for i in range(N_CORES):
        o2 = results[i]["out2"]
        parts.append(o2[0:OUT_DIM, :])
        parts.append(o2[OUT_DIM:, :])
    out_t = np.concatenate(parts, axis=1)  # [64, 500000]
    return np.ascontiguousarray(out_t.T).astype(np.float32)


def kernel(x: np.ndarray, weight_: np.ndarray) -> np.ndarray:
    x = np.asarray(x)
    weight_ = np.asarray(weight_)
    res = _run(_make_in_maps(x, weight_)).results
    return _assemble(res)
